# revision 4
# baseline (speedup 1.0000x reference)
"""Dual-stream attention kernel for TRN2 — one batch element per core (v2).

Per-core computation (batch element b):
  qb^T = Wq @ q_b^T          [C, N]   fp16, transposed layout (c on partitions)
  kb^T = Wk @ k_b^T          [C, N]   fp16
  vcomb[tb][tok, h, 0:64]   = (v_b @ Wv^T)    per-head slices   (natural layout)
  vcomb[tb][tok, h, 64:128] = (v_img_b @ Wvim^T)
  per head pair ct (2 heads = one 128-partition q/k tile):
    S^T = kh @ qh^T       K=64 matmuls, row-paired on the PE (lower/upper head)
    E = exp(S^T * scale)  fp16, no max subtraction (logits ~N(0, 0.31))
    U = [vh | vih]^T @ E  M=128: rows 0:64 x-stream, 64:128 img-stream
    Esum = sum_kb E_kb    DVE fp16 tree (PE-free rowsum accumulation)
    rp = ones^T @ Esum    ONE [K=128,M=128] matmul: rowsum already broadcast
                          across all 128 partitions (replaces the old M=1
                          rowsum matmuls AND the separate broadcast matmul)
    O = U * (1/rp)        fp16 recip; DMA partition-shifts for the two
                          misaligned halves (i_lo, x_up); normalize of group
                          g interleaved into group g+1's PE stream
  x    = merge(O_x)  @ Wp^T  + bp
  x_im = merge(O_im) @ Wpi^T + bpi

All matmul operands are fp16 (10-bit mantissa, ~5e-4 component error).
PSUM accumulation is fp32 throughout.

build_module(loop_n=N) wraps the body in a hardware For_i loop for wall-clock
timing (amortizes the ~60 ms axon dispatch overhead); timing is
data-independent.
"""

import numpy as np
import concourse.bass as bass
import concourse.tile as tile
from concourse import bacc, mybir

P = 128
NTOK = 1024
C = 768
H = 12
DH = 64
CT = C // P  # 6 c-tiles
TB = NTOK // P  # 8 token blocks
QH = 2  # qt halves
KB = 8  # kt blocks
NQ = 512
SCALE = DH**-0.5
F32 = mybir.dt.float32
F16 = mybir.dt.float16
EXP = mybir.ActivationFunctionType.Exp
MULT = mybir.AluOpType.mult
ADD = mybir.AluOpType.add

XNAMES = ("xq", "xk", "xv", "xvi")
WNAMES = ("wq", "wk", "wv", "wvi", "wp", "wpi")


def build_module(num_devices=8, loop_n=1, stages="123"):
    nc = bacc.Bacc(
        "TRN2", target_bir_lowering=False, debug=False, num_devices=num_devices
    )
    d = {}
    for nm in XNAMES:
        d[nm] = nc.dram_tensor(nm, [C, NTOK], F16, kind="ExternalInput").ap()
    for nm in WNAMES:
        d[nm] = nc.dram_tensor(nm, [C, C], F16, kind="ExternalInput").ap()
    d["ones"] = nc.dram_tensor("ones", [P, P], F16, kind="ExternalInput").ap()
    d["bp"] = nc.dram_tensor("bp", [P, C], F32, kind="ExternalInput").ap()
    d["bpi"] = nc.dram_tensor("bpi", [P, C], F32, kind="ExternalInput").ap()
    xo = nc.dram_tensor("xo", [NTOK, C], F32, kind="ExternalOutput").ap()
    xio = nc.dram_tensor("xio", [NTOK, C], F32, kind="ExternalOutput").ap()

    with tile.TileContext(nc) as tc:
        with (
            tc.tile_pool(name="persist", bufs=1) as pp,
            tc.tile_pool(name="wstage", bufs=2) as wpool,
            tc.tile_pool(name="xstage", bufs=2) as xpool,
            tc.tile_pool(name="wk", bufs=8) as wk,
            tc.tile_pool(name="nrm", bufs=8) as nrm,
            tc.tile_pool(name="ubp", bufs=24) as ubp,
            tc.tile_pool(name="tmp", bufs=8) as tmpp,
            tc.tile_pool(name="ps", bufs=8, space="PSUM") as psp,
        ):
            qbt = pp.tile([P, CT, NTOK], F16, tag="qbt")
            kbt = pp.tile([P, CT, NTOK], F16, tag="kbt")
            # [v | vi] per head: lhsT for the combined AV matmul
            vcomb = pp.tile([P, TB, H, P], F16, tag="vcomb")
            axt = pp.tile([P, CT, NTOK], F16, tag="axt")
            ait = pp.tile([P, CT, NTOK], F16, tag="ait")
            onest = pp.tile([P, P], F16, tag="onest")
            bpr = pp.tile([P, C], F32, tag="bpr")
            bpir = pp.tile([P, C], F32, tag="bpir")

            def stage1():
                nc.sync.dma_start(bpr[:], d["bp"])
                nc.sync.dma_start(bpir[:], d["bpi"])
                nc.sync.dma_start(onest[:], d["ones"])

                for src, wsrc, mode in (
                    ("xv", "wv", "nat_v"),
                    ("xvi", "wvi", "nat_vi"),
                    ("xq", "wq", "tr_q"),
                    ("xk", "wk", "tr_k"),
                ):
                    xt = xpool.tile([P, CT, NTOK], F16, tag="xt")
                    nc.sync.dma_start(
                        xt[:], d[src].rearrange("(ct p) n -> p ct n", p=P)
                    )
                    wt = wpool.tile([P, CT, C], F16, tag="wt")
                    nc.sync.dma_start(
                        wt[:], d[wsrc].rearrange("(ct p) c -> p ct c", p=P)
                    )
                    if mode.startswith("tr"):
                        dst = qbt if mode == "tr_q" else kbt
                        for co in range(CT):
                            for nh in range(QH):
                                ps = psp.tile([P, NQ], F32, tag="ps")
                                for ci in range(CT):
                                    nc.tensor.matmul(
                                        ps[:],
                                        wt[:, ci, co * P : (co + 1) * P],
                                        xt[:, ci, nh * NQ : (nh + 1) * NQ],
                                        start=(ci == 0),
                                        stop=(ci == CT - 1),
                                    )
                                nc.vector.tensor_copy(
                                    dst[:, co, nh * NQ : (nh + 1) * NQ], ps[:]
                                )
                    else:
                        off = 0 if mode == "nat_v" else DH
                        for tb in range(TB):
                            for c0, cw in ((0, 512), (512, 256)):
                                ps = psp.tile([P, NQ], F32, tag="ps")
                                for ci in range(CT):
                                    nc.tensor.matmul(
                                        ps[:, :cw],
                                        xt[:, ci, tb * P : (tb + 1) * P],
                                        wt[:, ci, c0 : c0 + cw],
                                        start=(ci == 0),
                                        stop=(ci == CT - 1),
                                    )
                                h0, h1 = c0 // DH, (c0 + cw) // DH
                                nc.vector.tensor_copy(
                                    vcomb[:, tb, h0:h1, off : off + DH],
                                    ps[:, :cw].rearrange("p (h dh) -> p h dh", dh=DH),
                                )

            def stage2():
                stash = []

                def normalize(item):
                    # One [K=128,M=128] ones-matmul per head: rowsum of Esum
                    # already broadcast across all 128 output partitions.
                    ct, qsl, ub_lo, ub_up, esum_lo, esum_up = item
                    for half, ub, esum in (
                        ("lo", ub_lo, esum_lo),
                        ("up", ub_up, esum_up),
                    ):
                        rp = psp.tile([P, NQ], F32, tag="ps")
                        nc.tensor.matmul(
                            rp[:], onest[:], esum[:], start=True, stop=True
                        )
                        rpinv = nrm.tile([P, NQ], F16, tag="rpinv")
                        with nc.allow_low_precision(reason="softmax recip fp16"):
                            nc.vector.reciprocal(rpinv[:], rp[:])
                        if half == "lo":
                            nc.vector.tensor_tensor(
                                axt[0:DH, ct, qsl], ub[0:DH, :], rpinv[0:DH, :],
                                MULT,
                            )
                            t_il = tmpp.tile([P, NQ], F16, tag="tshift")
                            nc.vector.tensor_tensor(
                                t_il[DH:P, :], ub[DH:P, :], rpinv[DH:P, :], MULT
                            )
                            nc.sync.dma_start(ait[0:DH, ct, qsl], t_il[DH:P, :])
                        else:
                            t_xu = tmpp.tile([P, NQ], F16, tag="tshift")
                            nc.vector.tensor_tensor(
                                t_xu[0:DH, :], ub[0:DH, :], rpinv[0:DH, :], MULT
                            )
                            nc.sync.dma_start(axt[DH:P, ct, qsl], t_xu[0:DH, :])
                            nc.vector.tensor_tensor(
                                ait[DH:P, ct, qsl], ub[DH:P, :], rpinv[DH:P, :],
                                MULT,
                            )

                for ct in range(CT):
                    h_lo, h_up = 2 * ct, 2 * ct + 1
                    for qh in range(QH):
                        qsl = slice(qh * NQ, (qh + 1) * NQ)
                        u_lo = psp.tile([P, NQ], F32, tag="ps")
                        u_up = psp.tile([P, NQ], F32, tag="ps")
                        esum_lo = nrm.tile([P, NQ], F16, tag="esum")
                        esum_up = nrm.tile([P, NQ], F16, tag="esum")
                        # software-pipelined: scores/exp run one kb ahead of
                        # the U consumers so the PE never waits on ACT; the
                        # rowsum accumulates on DVE (PE-free).
                        es = []
                        for kb in range(KB):
                            ksl = slice(kb * P, (kb + 1) * P)
                            s_lo = psp.tile([P, NQ], F32, tag="ps")
                            s_up = psp.tile([P, NQ], F32, tag="ps")
                            nc.tensor.matmul(
                                s_lo[:], kbt[0:DH, ct, ksl], qbt[0:DH, ct, qsl],
                                start=True, stop=True,
                            )
                            nc.tensor.matmul(
                                s_up[:], kbt[DH:P, ct, ksl], qbt[DH:P, ct, qsl],
                                start=True, stop=True,
                            )
                            e_lo = wk.tile([P, NQ], F16, tag="e")
                            e_up = wk.tile([P, NQ], F16, tag="e")
                            nc.scalar.activation(e_lo[:], s_lo[:], EXP, scale=SCALE)
                            nc.scalar.activation(e_up[:], s_up[:], EXP, scale=SCALE)
                            es.append((e_lo, e_up))
                            if kb == 1:
                                nc.vector.tensor_tensor(
                                    esum_lo[:], es[0][0][:], e_lo[:], ADD
                                )
                                nc.vector.tensor_tensor(
                                    esum_up[:], es[0][1][:], e_up[:], ADD
                                )
                            elif kb > 1:
                                nc.vector.tensor_tensor(
                                    esum_lo[:], esum_lo[:], e_lo[:], ADD
                                )
                                nc.vector.tensor_tensor(
                                    esum_up[:], esum_up[:], e_up[:], ADD
                                )
                            if kb > 0:
                                pe_lo, pe_up = es[kb - 1]
                                st, sp = kb - 1 == 0, False
                                pkb = kb - 1
                                nc.tensor.matmul(
                                    u_lo[:], vcomb[:, pkb, h_lo, :], pe_lo[:],
                                    start=st, stop=sp,
                                )
                                nc.tensor.matmul(
                                    u_up[:], vcomb[:, pkb, h_up, :], pe_up[:],
                                    start=st, stop=sp,
                                )
                        pe_lo, pe_up = es[KB - 1]
                        nc.tensor.matmul(
                            u_lo[:], vcomb[:, KB - 1, h_lo, :], pe_lo[:],
                            start=False, stop=True,
                        )
                        nc.tensor.matmul(
                            u_up[:], vcomb[:, KB - 1, h_up, :], pe_up[:],
                            start=False, stop=True,
                        )

                        # ---- drain PSUM fast (frees banks for the next group) ----
                        ub_lo = ubp.tile([P, NQ], F16, tag="ub")
                        ub_up = ubp.tile([P, NQ], F16, tag="ub")
                        nc.vector.tensor_copy(ub_lo[:], u_lo[:])
                        nc.vector.tensor_copy(ub_up[:], u_up[:])
                        stash.append((ct, qsl, ub_lo, ub_up, esum_lo, esum_up))
                        # normalize of the previous group rides inside this
                        # group's PE stream (plenty of slack) instead of a
                        # serialized tail pass.
                        if len(stash) > 1:
                            normalize(stash.pop(0))

                while stash:
                    normalize(stash.pop(0))

            def stage3():
                for dst_dram, src, w_nm, bias_t in (
                    (xo, axt, "wp", bpr),
                    (xio, ait, "wpi", bpir),
                ):
                    wt = wpool.tile([P, CT, C], F16, tag="wt")
                    nc.sync.dma_start(
                        wt[:], d[w_nm].rearrange("(ct p) c -> p ct c", p=P)
                    )
                    for tb in range(TB):
                        for c0, cw in ((0, 512), (512, 256)):
                            ps = psp.tile([P, NQ], F32, tag="ps")
                            for ci in range(CT):
                                nc.tensor.matmul(
                                    ps[:, :cw],
                                    src[:, ci, tb * P : (tb + 1) * P],
                                    wt[:, ci, c0 : c0 + cw],
                                    start=(ci == 0),
                                    stop=(ci == CT - 1),
                                )
                            ot = wk.tile([P, NQ], F32, tag="ot")
                            nc.vector.tensor_tensor(
                                ot[:, :cw], ps[:, :cw], bias_t[:, c0 : c0 + cw], ADD
                            )
                            nc.sync.dma_start(
                                dst_dram[tb * P : (tb + 1) * P, c0 : c0 + cw],
                                ot[:, :cw],
                            )

            def body():
                if "1" in stages:
                    stage1()
                if "2" in stages:
                    stage2()
                if "3" in stages:
                    stage3()

            if loop_n == 1:
                body()
            else:
                with tc.For_i(0, loop_n, 1):
                    body()

    nc.compile()
    return nc


def make_in_maps(q, k, v, v_img, Wq, Wk, Wv, Wvim, Wp, bp, Wpi, bpi, n_cores=8):
    """Host-side prep: per-core transposed fp16 activations + shared fp16 weights."""
    f = np.float32
    h = np.float16
    shared = {
        "wq": np.asarray(Wq, f).T.astype(h),
        "wk": np.asarray(Wk, f).T.astype(h),
        "wv": np.asarray(Wv, f).T.astype(h),
        "wvi": np.asarray(Wvim, f).T.astype(h),
        "wp": np.asarray(Wp, f).T.astype(h),
        "wpi": np.asarray(Wpi, f).T.astype(h),
        "ones": np.ones((P, P), h),
        "bp": np.ascontiguousarray(np.broadcast_to(np.asarray(bp, f), (P, C))),
        "bpi": np.ascontiguousarray(np.broadcast_to(np.asarray(bpi, f), (P, C))),
    }
    q = np.asarray(q, f)
    k = np.asarray(k, f)
    v = np.asarray(v, f)
    vi = np.asarray(v_img, f)
    in_maps = []
    for b in range(n_cores):
        in_maps.append(
            {
                "xq": np.ascontiguousarray(q[:, b, :].T).astype(h),
                "xk": np.ascontiguousarray(k[:, b, :].T).astype(h),
                "xv": np.ascontiguousarray(v[:, b, :].T).astype(h),
                "xvi": np.ascontiguousarray(vi[:, b, :].T).astype(h),
                **shared,
            }
        )
    return in_maps


# ---------------------------------------------------------------------------
# Harness entry point: full inputs in, full outputs out.
# Shards batch B=8 across the 8 NeuronCores (data parallel), no collectives.
# ---------------------------------------------------------------------------

_NC_CACHE = {}


def _get_module():
    if "nc" not in _NC_CACHE:
        _NC_CACHE["nc"] = build_module(num_devices=8)
    return _NC_CACHE["nc"]


def kernel(q, k, v, v_img, Wq, Wk, Wv, Wvim, Wp, bp, Wpi, bpi):
    from concourse.bass_utils import run_bass_kernel_spmd

    B = np.asarray(q).shape[1]
    nc = _get_module()
    in_maps = make_in_maps(q, k, v, v_img, Wq, Wk, Wv, Wvim, Wp, bp, Wpi, bpi,
                           n_cores=B)
    res = run_bass_kernel_spmd(nc, in_maps, core_ids=list(range(B)), trace=False)
    x = np.stack([res.results[b]["xo"] for b in range(B)])
    x_im = np.stack([res.results[b]["xio"] for b in range(B)])
    return (x, x_im)



# revision 6
# speedup vs baseline: 1.0150x; 1.0150x over previous
"""Dual-stream attention kernel for TRN2 — one batch element per core (v4).

Microbenchmark-calibrated design (real HW rates, not the nominal spec):
  PE fp16 matmul [*,512]:   ~311 ns   (~0.61 ns/moving-row effective)
  ACT exp  [128,512] psum:  ~772 ns   (~345 ns/instr fixed overhead)
  ACT exp  [128,1024] psum: ~1.14 us  (batching amortizes the overhead)
  DVE fp16 add [128,512]:   ~339 ns   (2x mode real)

Per-core computation (batch element b):
  qb^T = Wq @ q_b^T          [C, N]   fp16, transposed layout (c on partitions)
  kb^T = Wk @ k_b^T          [C, N]   fp16
  vcomb[tb][tok, h, 0:64]   = (v_b @ Wv^T)    per-head slices   (natural layout)
  vcomb[tb][tok, h, 64:128] = (v_img_b @ Wvim^T)
  per head pair ct (2 heads = one 128-partition q/k tile):
    S^T pair = kh @ qh^T   K=64 matmuls into one [128,2,512] PSUM pair
    E pair  = exp(S^T)     ONE 1024-wide ACT instr per kb (fp16 out, no max
                           subtraction; logits ~N(0, 0.31))
    U pair  = [vh|vih]^T @ E   both heads accumulate into one [128,2,512] PSUM
    Esum    = DVE pairwise tree over the 8 E pair tiles (7 adds, 1024-wide)
    rp pair = ones^T @ Esum    [K=128,M=128] matmuls: rowsum pre-broadcast
                           across all 128 partitions (no M=1 matmuls)
    O = U * (1/rp)         ONE 1024-wide fp16 recip; DMA partition-shifts for
                           the two misaligned halves; normalize of group g
                           rides inside group g+1's PE stream
  x    = merge(O_x)  @ Wp^T  + bp     (512|256 col splits share one PSUM pair,
  x_im = merge(O_im) @ Wpi^T + bpi     one 768-wide bias add + one DMA per tb)

All matmul operands are fp16 (10-bit mantissa, ~5e-4 component error).
PSUM accumulation is fp32 throughout.

build_module(loop_n=N) wraps the body in a hardware For_i loop for wall-clock
timing (amortizes the ~70 ms axon dispatch overhead); timing is
data-independent.
"""

import numpy as np
import concourse.bass as bass
import concourse.tile as tile
from concourse import bacc, mybir

P = 128
NTOK = 1024
C = 768
H = 12
DH = 64
CT = C // P  # 6 c-tiles
TB = NTOK // P  # 8 token blocks
QH = 2  # qt halves
KB = 8  # kt blocks
NQ = 512
SCALE = DH**-0.5
F32 = mybir.dt.float32
F16 = mybir.dt.float16
EXP = mybir.ActivationFunctionType.Exp
MULT = mybir.AluOpType.mult
ADD = mybir.AluOpType.add

XNAMES = ("xq", "xk", "xv", "xvi")
WNAMES = ("wq", "wk", "wv", "wvi", "wp", "wpi")


def build_module(num_devices=8, loop_n=1, stages="123"):
    nc = bacc.Bacc(
        "TRN2", target_bir_lowering=False, debug=False, num_devices=num_devices
    )
    d = {}
    for nm in XNAMES:
        d[nm] = nc.dram_tensor(nm, [C, NTOK], F16, kind="ExternalInput").ap()
    for nm in WNAMES:
        d[nm] = nc.dram_tensor(nm, [C, C], F16, kind="ExternalInput").ap()
    d["ones"] = nc.dram_tensor("ones", [P, P], F16, kind="ExternalInput").ap()
    d["bp"] = nc.dram_tensor("bp", [P, C], F32, kind="ExternalInput").ap()
    d["bpi"] = nc.dram_tensor("bpi", [P, C], F32, kind="ExternalInput").ap()
    xo = nc.dram_tensor("xo", [NTOK, C], F32, kind="ExternalOutput").ap()
    xio = nc.dram_tensor("xio", [NTOK, C], F32, kind="ExternalOutput").ap()

    with tile.TileContext(nc) as tc:
        with (
            tc.tile_pool(name="persist", bufs=1) as pp,
            tc.tile_pool(name="wstage", bufs=2) as wpool,
            tc.tile_pool(name="xstage", bufs=2) as xpool,
            tc.tile_pool(name="wk", bufs=8) as wk,
            tc.tile_pool(name="nrm", bufs=8) as nrm,
            tc.tile_pool(name="ubp", bufs=8) as ubp,
            tc.tile_pool(name="tmp", bufs=8) as tmpp,
            tc.tile_pool(name="ps", bufs=2, space="PSUM") as psp,
        ):
            qbt = pp.tile([P, CT, NTOK], F16, tag="qbt")
            kbt = pp.tile([P, CT, NTOK], F16, tag="kbt")
            # [v | vi] per head: lhsT for the combined AV matmul
            vcomb = pp.tile([P, TB, H, P], F16, tag="vcomb")
            axt = pp.tile([P, CT, NTOK], F16, tag="axt")
            ait = pp.tile([P, CT, NTOK], F16, tag="ait")
            onest = pp.tile([P, P], F16, tag="onest")
            bpr = pp.tile([P, C], F32, tag="bpr")
            bpir = pp.tile([P, C], F32, tag="bpir")

            def stage1():
                nc.sync.dma_start(bpr[:], d["bp"])
                nc.sync.dma_start(bpir[:], d["bpi"])
                nc.sync.dma_start(onest[:], d["ones"])

                for src, wsrc, mode in (
                    ("xv", "wv", "nat_v"),
                    ("xvi", "wvi", "nat_vi"),
                    ("xq", "wq", "tr_q"),
                    ("xk", "wk", "tr_k"),
                ):
                    xt = xpool.tile([P, CT, NTOK], F16, tag="xt")
                    nc.sync.dma_start(
                        xt[:], d[src].rearrange("(ct p) n -> p ct n", p=P)
                    )
                    wt = wpool.tile([P, CT, C], F16, tag="wt")
                    nc.sync.dma_start(
                        wt[:], d[wsrc].rearrange("(ct p) c -> p ct c", p=P)
                    )
                    if mode.startswith("tr"):
                        dst = qbt if mode == "tr_q" else kbt
                        for co in range(CT):
                            ps = psp.tile([P, QH, NQ], F32, tag="sp")
                            for nh in range(QH):
                                for ci in range(CT):
                                    nc.tensor.matmul(
                                        ps[:, nh, :],
                                        wt[:, ci, co * P : (co + 1) * P],
                                        xt[:, ci, nh * NQ : (nh + 1) * NQ],
                                        start=(ci == 0),
                                        stop=(ci == CT - 1),
                                    )
                            nc.vector.tensor_copy(
                                dst[:, co, :],
                                ps[:].rearrange("p a n -> p (a n)"),
                            )
                    else:
                        off = 0 if mode == "nat_v" else DH
                        for tb in range(TB):
                            ps = psp.tile([P, QH, NQ], F32, tag="sp")
                            for j, (c0, cw) in enumerate(((0, 512), (512, 256))):
                                for ci in range(CT):
                                    nc.tensor.matmul(
                                        ps[:, j, :cw],
                                        xt[:, ci, tb * P : (tb + 1) * P],
                                        wt[:, ci, c0 : c0 + cw],
                                        start=(ci == 0),
                                        stop=(ci == CT - 1),
                                    )
                            nc.vector.tensor_copy(
                                vcomb[:, tb, 0:8, off : off + DH],
                                ps[:, 0, :].rearrange("p (h dh) -> p h dh", dh=DH),
                            )
                            nc.vector.tensor_copy(
                                vcomb[:, tb, 8:12, off : off + DH],
                                ps[:, 1, 0:256].rearrange(
                                    "p (h dh) -> p h dh", dh=DH
                                ),
                            )

            def stage2():
                stash = []

                def normalize(item):
                    # [K=128,M=128] ones-matmuls: rowsum of Esum arrives
                    # already broadcast across output partitions.
                    ct, qsl, ub, z = item
                    rp = psp.tile([P, 2, NQ], F32, tag="u")
                    nc.tensor.matmul(
                        rp[:, 0, :], onest[:], z[:, 0, :], start=True, stop=True
                    )
                    nc.tensor.matmul(
                        rp[:, 1, :], onest[:], z[:, 1, :], start=True, stop=True
                    )
                    rpinv = nrm.tile([P, 2, NQ], F16, tag="rpinv", bufs=2)
                    with nc.allow_low_precision(reason="softmax recip fp16"):
                        nc.vector.reciprocal(rpinv[:], rp[:])
                    nc.vector.tensor_tensor(
                        axt[0:DH, ct, qsl], ub[0:DH, 0, :], rpinv[0:DH, 0, :], MULT
                    )
                    t_il = tmpp.tile([P, NQ], F16, tag="tshift")
                    nc.vector.tensor_tensor(
                        t_il[DH:P, :], ub[DH:P, 0, :], rpinv[DH:P, 0, :], MULT
                    )
                    nc.sync.dma_start(ait[0:DH, ct, qsl], t_il[DH:P, :])
                    t_xu = tmpp.tile([P, NQ], F16, tag="tshift")
                    nc.vector.tensor_tensor(
                        t_xu[0:DH, :], ub[0:DH, 1, :], rpinv[0:DH, 1, :], MULT
                    )
                    nc.sync.dma_start(axt[DH:P, ct, qsl], t_xu[0:DH, :])
                    nc.vector.tensor_tensor(
                        ait[DH:P, ct, qsl], ub[DH:P, 1, :], rpinv[DH:P, 1, :], MULT
                    )

                for ct in range(CT):
                    h_lo, h_up = 2 * ct, 2 * ct + 1
                    for qh in range(QH):
                        qsl = slice(qh * NQ, (qh + 1) * NQ)
                        u = psp.tile([P, 2, NQ], F32, tag="u")
                        # scores/exp run one kb ahead of the U consumers so
                        # the PE never waits on ACT; rowsum tree on DVE.
                        es = []
                        ts = []
                        z = None
                        for kb in range(KB):
                            ksl = slice(kb * P, (kb + 1) * P)
                            sp = psp.tile([P, 2, NQ], F32, tag="sp")
                            nc.tensor.matmul(
                                sp[:, 0, :], kbt[0:DH, ct, ksl], qbt[0:DH, ct, qsl],
                                start=True, stop=True,
                            )
                            nc.tensor.matmul(
                                sp[:, 1, :], kbt[DH:P, ct, ksl], qbt[DH:P, ct, qsl],
                                start=True, stop=True,
                            )
                            e = wk.tile([P, 2, NQ], F16, tag="e")
                            nc.scalar.activation(e[:], sp[:], EXP, scale=SCALE)
                            es.append(e)
                            if kb % 2 == 1:
                                t = nrm.tile([P, 2, NQ], F16, tag="tr", bufs=6)
                                nc.vector.tensor_tensor(
                                    t[:], es[kb - 1][:], es[kb][:], ADD
                                )
                                ts.append(t)
                                if kb == 3:
                                    q1 = nrm.tile([P, 2, NQ], F16, tag="tr", bufs=6)
                                    nc.vector.tensor_tensor(
                                        q1[:], ts[0][:], ts[1][:], ADD
                                    )
                                    ts.append(q1)
                                elif kb == 7:
                                    q2 = nrm.tile([P, 2, NQ], F16, tag="tr", bufs=6)
                                    nc.vector.tensor_tensor(
                                        q2[:], ts[2][:], ts[3][:], ADD
                                    )
                                    z = nrm.tile([P, 2, NQ], F16, tag="z", bufs=2)
                                    nc.vector.tensor_tensor(
                                        z[:], ts[4][:], q2[:], ADD
                                    )
                            if kb > 0:
                                pe = es[kb - 1]
                                st, sp_ = kb - 1 == 0, False
                                pkb = kb - 1
                                nc.tensor.matmul(
                                    u[:, 0, :], vcomb[:, pkb, h_lo, :], pe[:, 0, :],
                                    start=st, stop=sp_,
                                )
                                nc.tensor.matmul(
                                    u[:, 1, :], vcomb[:, pkb, h_up, :], pe[:, 1, :],
                                    start=st, stop=sp_,
                                )
                        pe = es[KB - 1]
                        nc.tensor.matmul(
                            u[:, 0, :], vcomb[:, KB - 1, h_lo, :], pe[:, 0, :],
                            start=False, stop=True,
                        )
                        nc.tensor.matmul(
                            u[:, 1, :], vcomb[:, KB - 1, h_up, :], pe[:, 1, :],
                            start=False, stop=True,
                        )

                        # ---- drain PSUM fast (frees the bank pair) ----
                        ub = ubp.tile([P, 2, NQ], F16, tag="ub")
                        nc.vector.tensor_copy(ub[:], u[:])
                        stash.append((ct, qsl, ub, z))
                        # normalize of the previous group rides inside this
                        # group's PE stream instead of a serialized tail.
                        if len(stash) > 1:
                            normalize(stash.pop(0))

                while stash:
                    normalize(stash.pop(0))

            def stage3():
                for dst_dram, src, w_nm, bias_t in (
                    (xo, axt, "wp", bpr),
                    (xio, ait, "wpi", bpir),
                ):
                    wt = wpool.tile([P, CT, C], F16, tag="wt")
                    nc.sync.dma_start(
                        wt[:], d[w_nm].rearrange("(ct p) c -> p ct c", p=P)
                    )
                    for tb in range(TB):
                        ps = psp.tile([P, QH, NQ], F32, tag="sp")
                        for j, (c0, cw) in enumerate(((0, 512), (512, 256))):
                            for ci in range(CT):
                                nc.tensor.matmul(
                                    ps[:, j, :cw],
                                    src[:, ci, tb * P : (tb + 1) * P],
                                    wt[:, ci, c0 : c0 + cw],
                                    start=(ci == 0),
                                    stop=(ci == CT - 1),
                                )
                        ot = wk.tile([P, C], F32, tag="ot", bufs=4)
                        nc.vector.tensor_tensor(
                            ot[:, 0:512], ps[:, 0, :], bias_t[:, 0:512], ADD
                        )
                        nc.vector.tensor_tensor(
                            ot[:, 512:768], ps[:, 1, 0:256], bias_t[:, 512:768], ADD
                        )
                        nc.sync.dma_start(
                            dst_dram[tb * P : (tb + 1) * P, :], ot[:]
                        )

            def body():
                if "1" in stages:
                    stage1()
                if "2" in stages:
                    stage2()
                if "3" in stages:
                    stage3()

            if loop_n == 1:
                body()
            else:
                with tc.For_i(0, loop_n, 1):
                    body()

    nc.compile()
    return nc


def make_in_maps(q, k, v, v_img, Wq, Wk, Wv, Wvim, Wp, bp, Wpi, bpi, n_cores=8):
    """Host-side prep: per-core transposed fp16 activations + shared fp16 weights."""
    f = np.float32
    h = np.float16
    shared = {
        "wq": np.asarray(Wq, f).T.astype(h),
        "wk": np.asarray(Wk, f).T.astype(h),
        "wv": np.asarray(Wv, f).T.astype(h),
        "wvi": np.asarray(Wvim, f).T.astype(h),
        "wp": np.asarray(Wp, f).T.astype(h),
        "wpi": np.asarray(Wpi, f).T.astype(h),
        "ones": np.ones((P, P), h),
        "bp": np.ascontiguousarray(np.broadcast_to(np.asarray(bp, f), (P, C))),
        "bpi": np.ascontiguousarray(np.broadcast_to(np.asarray(bpi, f), (P, C))),
    }
    q = np.asarray(q, f)
    k = np.asarray(k, f)
    v = np.asarray(v, f)
    vi = np.asarray(v_img, f)
    in_maps = []
    for b in range(n_cores):
        in_maps.append(
            {
                "xq": np.ascontiguousarray(q[:, b, :].T).astype(h),
                "xk": np.ascontiguousarray(k[:, b, :].T).astype(h),
                "xv": np.ascontiguousarray(v[:, b, :].T).astype(h),
                "xvi": np.ascontiguousarray(vi[:, b, :].T).astype(h),
                **shared,
            }
        )
    return in_maps


# ---------------------------------------------------------------------------
# Harness entry point: full inputs in, full outputs out.
# Shards batch B=8 across the 8 NeuronCores (data parallel), no collectives.
# ---------------------------------------------------------------------------

_NC_CACHE = {}


def _get_module():
    if "nc" not in _NC_CACHE:
        _NC_CACHE["nc"] = build_module(num_devices=8)
    return _NC_CACHE["nc"]


def kernel(q, k, v, v_img, Wq, Wk, Wv, Wvim, Wp, bp, Wpi, bpi):
    from concourse.bass_utils import run_bass_kernel_spmd

    B = np.asarray(q).shape[1]
    nc = _get_module()
    in_maps = make_in_maps(q, k, v, v_img, Wq, Wk, Wv, Wvim, Wp, bp, Wpi, bpi,
                           n_cores=B)
    res = run_bass_kernel_spmd(nc, in_maps, core_ids=list(range(B)), trace=False)
    x = np.stack([res.results[b]["xo"] for b in range(B)])
    x_im = np.stack([res.results[b]["xio"] for b in range(B)])
    return (x, x_im)


# revision 8
# speedup vs baseline: 1.0955x; 1.0792x over previous
"""Dual-stream attention kernel for TRN2 — one batch element per core (v4).

Microbenchmark-calibrated design (real HW rates, not the nominal spec):
  PE fp16 matmul [*,512]:   ~311 ns   (~0.61 ns/moving-row effective)
  ACT exp  [128,512] psum:  ~772 ns   (~345 ns/instr fixed overhead)
  ACT exp  [128,1024] psum: ~1.14 us  (batching amortizes the overhead)
  DVE fp16 add [128,512]:   ~339 ns   (2x mode real)

Per-core computation (batch element b):
  qb^T = Wq @ q_b^T          [C, N]   fp16, transposed layout (c on partitions)
  kb^T = Wk @ k_b^T          [C, N]   fp16
  vcomb[tb][tok, h, 0:64]   = (v_b @ Wv^T)    per-head slices   (natural layout)
  vcomb[tb][tok, h, 64:128] = (v_img_b @ Wvim^T)
  per head pair ct (2 heads = one 128-partition q/k tile):
    S^T pair = kh @ qh^T   K=64 matmuls into one [128,2,512] PSUM pair
    E pair  = exp(S^T)     ONE 1024-wide ACT instr per kb (fp16 out, no max
                           subtraction; logits ~N(0, 0.31))
    U pair  = [vh|vih]^T @ E   both heads accumulate into one [128,2,512] PSUM
    Esum    = DVE pairwise tree over the 8 E pair tiles (7 adds, 1024-wide)
    rp pair = ones^T @ Esum    [K=128,M=128] matmuls: rowsum pre-broadcast
                           across all 128 partitions (no M=1 matmuls)
    O = U * (1/rp)         ONE 1024-wide fp16 recip; DMA partition-shifts for
                           the two misaligned halves; normalize of group g
                           rides inside group g+1's PE stream
  x    = merge(O_x)  @ Wp^T  + bp     (512|256 col splits share one PSUM pair,
  x_im = merge(O_im) @ Wpi^T + bpi     one 768-wide bias add + one DMA per tb)

All matmul operands are fp16 (10-bit mantissa, ~5e-4 component error).
PSUM accumulation is fp32 throughout.

build_module(loop_n=N) wraps the body in a hardware For_i loop for wall-clock
timing (amortizes the ~70 ms axon dispatch overhead); timing is
data-independent.
"""

import numpy as np
import concourse.bass as bass
import concourse.tile as tile
from concourse import bacc, mybir

P = 128
NTOK = 1024
C = 768
H = 12
DH = 64
CT = C // P  # 6 c-tiles
TB = NTOK // P  # 8 token blocks
QH = 2  # qt halves
KB = 8  # kt blocks
NQ = 512
SCALE = DH**-0.5
F32 = mybir.dt.float32
F16 = mybir.dt.float16
EXP = mybir.ActivationFunctionType.Exp
IDENT = mybir.ActivationFunctionType.Identity
MULT = mybir.AluOpType.mult
ADD = mybir.AluOpType.add

XNAMES = ("xq", "xk", "xv", "xvi")
WNAMES = ("wq", "wk", "wv", "wvi", "wp", "wpi")


def build_module(num_devices=8, loop_n=1, stages="123"):
    nc = bacc.Bacc(
        "TRN2", target_bir_lowering=False, debug=False, num_devices=num_devices
    )
    d = {}
    for nm in XNAMES:
        d[nm] = nc.dram_tensor(nm, [C, NTOK], F16, kind="ExternalInput").ap()
    for nm in WNAMES:
        d[nm] = nc.dram_tensor(nm, [C, C], F16, kind="ExternalInput").ap()
    d["ones"] = nc.dram_tensor("ones", [P, P], F16, kind="ExternalInput").ap()
    d["bpt"] = nc.dram_tensor("bpt", [P, CT], F32, kind="ExternalInput").ap()
    d["bpit"] = nc.dram_tensor("bpit", [P, CT], F32, kind="ExternalInput").ap()
    xo = nc.dram_tensor("xo", [C, NTOK], F16, kind="ExternalOutput").ap()
    xio = nc.dram_tensor("xio", [C, NTOK], F16, kind="ExternalOutput").ap()

    with tile.TileContext(nc) as tc:
        with (
            tc.tile_pool(name="persist", bufs=1) as pp,
            tc.tile_pool(name="wstage", bufs=2) as wpool,
            tc.tile_pool(name="xstage", bufs=2) as xpool,
            tc.tile_pool(name="wk", bufs=8) as wk,
            tc.tile_pool(name="nrm", bufs=8) as nrm,
            tc.tile_pool(name="ubp", bufs=8) as ubp,
            tc.tile_pool(name="tmp", bufs=8) as tmpp,
            tc.tile_pool(name="ps", bufs=2, space="PSUM") as psp,
        ):
            qbt = pp.tile([P, CT, NTOK], F16, tag="qbt")
            kbt = pp.tile([P, CT, NTOK], F16, tag="kbt")
            # [v | vi] per head: lhsT for the combined AV matmul
            vcomb = pp.tile([P, TB, H, P], F16, tag="vcomb")
            axt = pp.tile([P, CT, NTOK], F16, tag="axt")
            ait = pp.tile([P, CT, NTOK], F16, tag="ait")
            onest = pp.tile([P, P], F16, tag="onest")
            bptt = pp.tile([P, CT], F32, tag="bptt")
            bpitt = pp.tile([P, CT], F32, tag="bpitt")

            def stage1():
                nc.sync.dma_start(bptt[:], d["bpt"])
                nc.sync.dma_start(bpitt[:], d["bpit"])
                nc.sync.dma_start(onest[:], d["ones"])

                for src, wsrc, mode in (
                    ("xv", "wv", "nat_v"),
                    ("xvi", "wvi", "nat_vi"),
                    ("xq", "wq", "tr_q"),
                    ("xk", "wk", "tr_k"),
                ):
                    xt = xpool.tile([P, CT, NTOK], F16, tag="xt")
                    nc.sync.dma_start(
                        xt[:], d[src].rearrange("(ct p) n -> p ct n", p=P)
                    )
                    wt = wpool.tile([P, CT, C], F16, tag="wt")
                    nc.sync.dma_start(
                        wt[:], d[wsrc].rearrange("(ct p) c -> p ct c", p=P)
                    )
                    if mode.startswith("tr"):
                        for co in range(CT):
                            ps = psp.tile([P, QH, NQ], F32, tag="sp")
                            for nh in range(QH):
                                for ci in range(CT):
                                    nc.tensor.matmul(
                                        ps[:, nh, :],
                                        wt[:, ci, co * P : (co + 1) * P],
                                        xt[:, ci, nh * NQ : (nh + 1) * NQ],
                                        start=(ci == 0),
                                        stop=(ci == CT - 1),
                                    )
                            dst = qbt if mode == "tr_q" else kbt
                            nc.vector.tensor_copy(
                                dst[:, co, :],
                                ps[:].rearrange("p a n -> p (a n)"),
                            )
                    else:
                        off = 0 if mode == "nat_v" else DH
                        for tb in range(TB):
                            ps = psp.tile([P, QH, NQ], F32, tag="sp")
                            for j, (c0, cw) in enumerate(((0, 512), (512, 256))):
                                for ci in range(CT):
                                    nc.tensor.matmul(
                                        ps[:, j, :cw],
                                        xt[:, ci, tb * P : (tb + 1) * P],
                                        wt[:, ci, c0 : c0 + cw],
                                        start=(ci == 0),
                                        stop=(ci == CT - 1),
                                    )
                            nc.vector.tensor_copy(
                                vcomb[:, tb, 0:8, off : off + DH],
                                ps[:, 0, :].rearrange("p (h dh) -> p h dh", dh=DH),
                            )
                            nc.vector.tensor_copy(
                                vcomb[:, tb, 8:12, off : off + DH],
                                ps[:, 1, 0:256].rearrange(
                                    "p (h dh) -> p h dh", dh=DH
                                ),
                            )

            def stage2():
                stash = []

                def normalize(item):
                    # [K=128,M=128] ones-matmuls: rowsum of Esum arrives
                    # already broadcast across output partitions.
                    ct, qsl, ub, z = item
                    rp = psp.tile([P, 2, NQ], F32, tag="u")
                    nc.tensor.matmul(
                        rp[:, 0, :], onest[:], z[:, 0, :], start=True, stop=True
                    )
                    nc.tensor.matmul(
                        rp[:, 1, :], onest[:], z[:, 1, :], start=True, stop=True
                    )
                    rpinv = nrm.tile([P, 2, NQ], F16, tag="rpinv", bufs=2)
                    with nc.allow_low_precision(reason="softmax recip fp16"):
                        nc.vector.reciprocal(rpinv[:], rp[:])
                    nc.vector.tensor_tensor(
                        axt[0:DH, ct, qsl], ub[0:DH, 0, :], rpinv[0:DH, 0, :], MULT
                    )
                    t_il = tmpp.tile([P, NQ], F16, tag="tshift")
                    nc.vector.tensor_tensor(
                        t_il[DH:P, :], ub[DH:P, 0, :], rpinv[DH:P, 0, :], MULT
                    )
                    nc.sync.dma_start(ait[0:DH, ct, qsl], t_il[DH:P, :])
                    t_xu = tmpp.tile([P, NQ], F16, tag="tshift")
                    nc.vector.tensor_tensor(
                        t_xu[0:DH, :], ub[0:DH, 1, :], rpinv[0:DH, 1, :], MULT
                    )
                    nc.sync.dma_start(axt[DH:P, ct, qsl], t_xu[0:DH, :])
                    nc.vector.tensor_tensor(
                        ait[DH:P, ct, qsl], ub[DH:P, 1, :], rpinv[DH:P, 1, :], MULT
                    )

                for ct in range(CT):
                    h_lo, h_up = 2 * ct, 2 * ct + 1
                    for qh in range(QH):
                        qsl = slice(qh * NQ, (qh + 1) * NQ)
                        u = psp.tile([P, 2, NQ], F32, tag="u")
                        # scores/exp run one kb ahead of the U consumers so
                        # the PE never waits on ACT; rowsum tree on DVE.
                        es = []
                        ts = []
                        z = None
                        for kb in range(KB):
                            ksl = slice(kb * P, (kb + 1) * P)
                            sp = psp.tile([P, 2, NQ], F32, tag="sp")
                            nc.tensor.matmul(
                                sp[:, 0, :], kbt[0:DH, ct, ksl], qbt[0:DH, ct, qsl],
                                start=True, stop=True,
                            )
                            nc.tensor.matmul(
                                sp[:, 1, :], kbt[DH:P, ct, ksl], qbt[DH:P, ct, qsl],
                                start=True, stop=True,
                            )
                            e = wk.tile([P, 2, NQ], F16, tag="e")
                            nc.scalar.activation(e[:], sp[:], EXP, scale=SCALE)
                            es.append(e)
                            if kb % 2 == 1:
                                t = nrm.tile([P, 2, NQ], F16, tag="tr", bufs=6)
                                nc.vector.tensor_tensor(
                                    t[:], es[kb - 1][:], es[kb][:], ADD
                                )
                                ts.append(t)
                                if kb == 3:
                                    q1 = nrm.tile([P, 2, NQ], F16, tag="tr", bufs=6)
                                    nc.vector.tensor_tensor(
                                        q1[:], ts[0][:], ts[1][:], ADD
                                    )
                                    ts.append(q1)
                                elif kb == 7:
                                    q2 = nrm.tile([P, 2, NQ], F16, tag="tr", bufs=6)
                                    nc.vector.tensor_tensor(
                                        q2[:], ts[2][:], ts[3][:], ADD
                                    )
                                    z = nrm.tile([P, 2, NQ], F16, tag="z", bufs=2)
                                    nc.vector.tensor_tensor(
                                        z[:], ts[4][:], q2[:], ADD
                                    )
                            if kb > 0:
                                pe = es[kb - 1]
                                st, sp_ = kb - 1 == 0, False
                                pkb = kb - 1
                                nc.tensor.matmul(
                                    u[:, 0, :], vcomb[:, pkb, h_lo, :], pe[:, 0, :],
                                    start=st, stop=sp_,
                                )
                                nc.tensor.matmul(
                                    u[:, 1, :], vcomb[:, pkb, h_up, :], pe[:, 1, :],
                                    start=st, stop=sp_,
                                )
                        pe = es[KB - 1]
                        nc.tensor.matmul(
                            u[:, 0, :], vcomb[:, KB - 1, h_lo, :], pe[:, 0, :],
                            start=False, stop=True,
                        )
                        nc.tensor.matmul(
                            u[:, 1, :], vcomb[:, KB - 1, h_up, :], pe[:, 1, :],
                            start=False, stop=True,
                        )

                        # ---- drain PSUM fast (frees the bank pair) ----
                        ub = ubp.tile([P, 2, NQ], F16, tag="ub")
                        nc.vector.tensor_copy(ub[:], u[:])
                        stash.append((ct, qsl, ub, z))
                        # normalize of the previous group rides inside this
                        # group's PE stream instead of a serialized tail.
                        if len(stash) > 1:
                            normalize(stash.pop(0))

                while stash:
                    normalize(stash.pop(0))

            def stage3():
                # Transposed outputs x^T [C, NTOK] fp16 (host untransposes):
                # same 72-matmul shape as the stage1 q/k projections (vs 96
                # in token-major layout), bias fused into the ACT drain.
                for dst_dram, srct, w_nm, bias_t in (
                    (xo, axt, "wp", bptt),
                    (xio, ait, "wpi", bpitt),
                ):
                    wt = wpool.tile([P, CT, C], F16, tag="wt")
                    nc.sync.dma_start(
                        wt[:], d[w_nm].rearrange("(ct p) c -> p ct c", p=P)
                    )
                    for co in range(CT):
                        ps = psp.tile([P, QH, NQ], F32, tag="sp")
                        for nh in range(QH):
                            for ci in range(CT):
                                nc.tensor.matmul(
                                    ps[:, nh, :],
                                    wt[:, ci, co * P : (co + 1) * P],
                                    srct[:, ci, nh * NQ : (nh + 1) * NQ],
                                    start=(ci == 0),
                                    stop=(ci == CT - 1),
                                )
                        ot = wk.tile([P, QH, NQ], F16, tag="ot", bufs=4)
                        nc.scalar.activation(
                            ot[:], ps[:], IDENT,
                            bias=bias_t[:, co : co + 1], scale=1.0,
                        )
                        nc.sync.dma_start(
                            dst_dram[co * P : (co + 1) * P, :],
                            ot[:].rearrange("p a n -> p (a n)"),
                        )

            def body():
                if "1" in stages:
                    stage1()
                if "2" in stages:
                    stage2()
                if "3" in stages:
                    stage3()

            if loop_n == 1:
                body()
            else:
                with tc.For_i(0, loop_n, 1):
                    body()

    nc.compile()
    return nc


def make_in_maps(q, k, v, v_img, Wq, Wk, Wv, Wvim, Wp, bp, Wpi, bpi, n_cores=8):
    """Host-side prep: per-core transposed fp16 activations + shared fp16 weights."""
    f = np.float32
    h = np.float16
    shared = {
        "wq": np.asarray(Wq, f).T.astype(h),
        "wk": np.asarray(Wk, f).T.astype(h),
        "wv": np.asarray(Wv, f).T.astype(h),
        "wvi": np.asarray(Wvim, f).T.astype(h),
        "wp": np.asarray(Wp, f).T.astype(h),
        "wpi": np.asarray(Wpi, f).T.astype(h),
        "ones": np.ones((P, P), h),
        "bpt": np.ascontiguousarray(np.asarray(bp, f).reshape(CT, P).T),
        "bpit": np.ascontiguousarray(np.asarray(bpi, f).reshape(CT, P).T),
    }
    q = np.asarray(q, f)
    k = np.asarray(k, f)
    v = np.asarray(v, f)
    vi = np.asarray(v_img, f)
    in_maps = []
    for b in range(n_cores):
        in_maps.append(
            {
                "xq": np.ascontiguousarray(q[:, b, :].T).astype(h),
                "xk": np.ascontiguousarray(k[:, b, :].T).astype(h),
                "xv": np.ascontiguousarray(v[:, b, :].T).astype(h),
                "xvi": np.ascontiguousarray(vi[:, b, :].T).astype(h),
                **shared,
            }
        )
    return in_maps


# ---------------------------------------------------------------------------
# Harness entry point: full inputs in, full outputs out.
# Shards batch B=8 across the 8 NeuronCores (data parallel), no collectives.
# ---------------------------------------------------------------------------

_NC_CACHE = {}


def _get_module():
    if "nc" not in _NC_CACHE:
        _NC_CACHE["nc"] = build_module(num_devices=8)
    return _NC_CACHE["nc"]


def kernel(q, k, v, v_img, Wq, Wk, Wv, Wvim, Wp, bp, Wpi, bpi):
    from concourse.bass_utils import run_bass_kernel_spmd

    B = np.asarray(q).shape[1]
    nc = _get_module()
    in_maps = make_in_maps(q, k, v, v_img, Wq, Wk, Wv, Wvim, Wp, bp, Wpi, bpi,
                           n_cores=B)
    res = run_bass_kernel_spmd(nc, in_maps, core_ids=list(range(B)), trace=False)
    x = np.stack([res.results[b]["xo"].T.astype(np.float32) for b in range(B)])
    x_im = np.stack(
        [res.results[b]["xio"].T.astype(np.float32) for b in range(B)]
    )
    return (x, x_im)
